# revision 14
# baseline (speedup 1.0000x reference)
"""Bass/Tile TRN2 kernel for nn_CTransformerBlock (point-transformer block).

Self-contained: hardcodes shapes B=4, N=2048, dp=32, dm=128, K=36.
Sharding: 8 cores = (batch b, half h); core 2b+h handles points
[h*1024, (h+1)*1024) of batch b. All per-core divergence is carried by
host-sliced inputs; the NEFF is SPMD-uniform.

Exactness: knn indices must match jax-on-neuron top_k bit-for-bit (the
point cloud has near-duplicate clusters). Verified recipe: K=3 fp32 PE
matmul for the xyz dot, sequential-f32 sq, ACT-Identity + DVE
scalar_tensor_tensor for (2*dot - (sq_n + sq_m)), then 5 rounds of
DVE max/max_index/match_replace (stable, lower-index-first).
"""
import numpy as np

import concourse.bacc as bacc
import concourse.bass as bass
import concourse.mybir as mybir
import concourse.tile as tile
from concourse import bass_utils
from concourse.alu_op_type import AluOpType

F32 = mybir.dt.float32
I16 = mybir.dt.int16
U32 = mybir.dt.uint32
AF = mybir.ActivationFunctionType

B = 4
N = 2048
DP = 32
DM = 128
K = 36
NT = 8              # point tiles per core (1024 points)
NLOC = NT * 128
NEG_INF = -3.0e38
KT = 12             # k-slices per gather third
GRP = 4             # k-slices per compute group

_CACHE = {}


def _build():
    nc = bacc.Bacc("TRN2", target_bir_lowering=False, debug=False)

    def inp(name, shape):
        return nc.dram_tensor(name, shape, F32, kind="ExternalInput")

    xyzT_d = inp("xyzT", [3, N])
    xyzTl_d = inp("xyzTl", [3, NLOC])
    sq_row_d = inp("sq_row", [1, N])
    featT_d = inp("featT", [DP, N])
    featTl_d = inp("featTl", [DP, NLOC])
    fc1_wT_d = inp("fc1_wT", [DP, DM])
    dw1_negT_d = inp("dw1_negT", [3, DM])
    wmats_d = inp("wmats", [DM, 9 * DM])
    wcols_d = inp("wcols", [DM, 16])
    wrows_d = inp("wrows", [1, 3 * DM])
    cw2T_d = inp("cw2T", [DM, DM * DM])
    cw3T_d = inp("cw3T", [DM, DM * DM])
    Cw_d = inp("Cw", [8, 128, 128])

    attn_o = nc.dram_tensor("attn_o", [NLOC, K, DM], F32, kind="ExternalOutput")
    res_o = nc.dram_tensor("res_o", [NLOC, DM], F32, kind="ExternalOutput")

    mf2_in = nc.dram_tensor("mf2_in", [DM, B], F32)
    mf2_out = nc.dram_tensor("mf2_out", [DM, B], F32, addr_space="Shared")
    mf3_in = nc.dram_tensor("mf3_in", [DM, B], F32)
    mf3_out = nc.dram_tensor("mf3_out", [DM, B], F32, addr_space="Shared")

    with tile.TileContext(nc) as tc:
        with (
            tc.tile_pool(name="cons", bufs=1) as cons,
            tc.tile_pool(name="sml", bufs=2) as sml,
            tc.tile_pool(name="med", bufs=2) as med,
            tc.tile_pool(name="gat", bufs=2) as gat,
            tc.tile_pool(name="gat1", bufs=1) as gat1,
            tc.tile_pool(name="med1", bufs=1) as med1,
            tc.tile_pool(name="ps", bufs=4, space="PSUM") as ps,
            tc.tile_pool(name="psr", bufs=1, space="PSUM") as psr,
        ):
            def load(dram):
                t = cons.tile(dram.shape, F32, tag=dram.name + "_s")
                nc.sync.dma_start(t[:], dram[:])
                return t

            xyzT = load(xyzT_d)
            xyzTl = load(xyzTl_d)
            fc1_wT = load(fc1_wT_d)
            dw1_negT = load(dw1_negT_d)
            wmats = load(wmats_d)
            g1k_negT = wmats[:, 0 * DM:1 * DM]
            g1qT = wmats[:, 1 * DM:2 * DM]
            wvT = wmats[:, 2 * DM:3 * DM]
            PdT = wmats[:, 3 * DM:4 * DM]
            d2T = wmats[:, 4 * DM:5 * DM]
            g2T = wmats[:, 5 * DM:6 * DM]
            cb2 = wmats[:, 6 * DM:7 * DM]
            cb3 = wmats[:, 7 * DM:8 * DM]
            I128 = wmats[:, 8 * DM:9 * DM]
            wcols = load(wcols_d)
            fc1_b = wcols[:, 0:1]
            qc0 = wcols[:, 1:2]
            b1d = wcols[:, 2:3]
            sqmT = wcols[:, 3:3 + NT]
            pmask = wcols[:, 11:11 + B]
            wrows = load(wrows_d)
            b2d_row = wrows[:, 0:DM]
            b2g_row = wrows[:, DM:2 * DM]
            ones1 = wrows[:, 2 * DM:3 * DM]
            Cw = cons.tile([128, 8 * 128], F32)
            for a in range(8):
                nc.sync.dma_start(Cw[:, a * 128:(a + 1) * 128], Cw_d[a])

            # ---------------- prep ----------------
            # sqn_rep: broadcast sq row to all partitions straight from DRAM
            sqn_rep = cons.tile([128, N], F32)
            nc.sync.dma_start(sqn_rep[:], sq_row_d[0:1, :].to_broadcast([128, N]))

            x_cm = med.tile([DM, N], F32, tag="big")
            for c in range(4):
                stg = sml.tile([DP, 512], F32, tag="ftstg")
                nc.sync.dma_start(stg[:], featT_d[:, c * 512:(c + 1) * 512])
                p = ps.tile([128, 512], F32, tag="ps")
                nc.tensor.matmul(p[:], fc1_wT[:], stg[:], start=True, stop=True)
                nc.scalar.activation(x_cm[:, c * 512:(c + 1) * 512], p[:],
                                     AF.Identity, bias=fc1_b)
            x_loc = cons.tile([DM, NLOC], F32)
            for c in range(2):
                stg = sml.tile([DP, 512], F32, tag="ftstg")
                nc.sync.dma_start(stg[:], featTl_d[:, c * 512:(c + 1) * 512])
                p = ps.tile([128, 512], F32, tag="ps")
                nc.tensor.matmul(p[:], fc1_wT[:], stg[:], start=True, stop=True)
                nc.scalar.activation(x_loc[:, c * 512:(c + 1) * 512], p[:],
                                     AF.Identity, bias=fc1_b)

            # gather sources: kge_src[...,0]=kgneg, [...,1]=eneg; v_src=v
            kge_src = cons.tile([128, N, 2], F32)
            v_src = cons.tile([128, N], F32)
            for c in range(4):
                sl = slice(c * 512, (c + 1) * 512)
                p = ps.tile([128, 512], F32, tag="ps")
                nc.tensor.matmul(p[:], g1k_negT, x_cm[:, sl], start=True, stop=True)
                nc.vector.tensor_copy(kge_src[:, sl, 0], p[:])
                p2 = ps.tile([128, 512], F32, tag="ps")
                nc.tensor.matmul(p2[:], dw1_negT[:], xyzT[:, sl], start=True, stop=True)
                nc.vector.tensor_copy(kge_src[:, sl, 1], p2[:])
                p3 = ps.tile([128, 512], F32, tag="ps")
                nc.tensor.matmul(p3[:], wvT, x_cm[:, sl], start=True, stop=True)
                nc.vector.tensor_copy(v_src[:, sl], p3[:])

            qgp = cons.tile([DM, NLOC], F32)
            ep = cons.tile([DM, NLOC], F32)
            for c in range(2):
                sl = slice(c * 512, (c + 1) * 512)
                p = ps.tile([128, 512], F32, tag="ps")
                nc.tensor.matmul(p[:], g1qT, x_loc[:, sl], start=True, stop=True)
                nc.scalar.activation(qgp[:, sl], p[:], AF.Identity, bias=qc0)
                p2 = ps.tile([128, 512], F32, tag="ps")
                nc.tensor.matmul(p2[:], dw1_negT[:], xyzTl[:, sl], start=True, stop=True)
                nc.scalar.activation(ep[:, sl], p2[:], AF.Identity,
                                     bias=b1d, scale=-1.0)

            res1_cm = cons.tile([DM, NLOC], F32)

            # ---------------- per-tile: dists/topk/wrap/attention ----------------
            for t in range(NT):
                tsl = slice(t * 128, (t + 1) * 128)

                negd = med.tile([128, N], F32, tag="big")
                for c in range(4):
                    sl = slice(c * 512, (c + 1) * 512)
                    pd = ps.tile([128, 512], F32, tag="ps")
                    nc.tensor.matmul(pd[:], xyzTl[:, tsl], xyzT[:, sl],
                                     start=True, stop=True)
                    trow = sml.tile([128, 512], F32, tag="trow")
                    nc.scalar.activation(trow[:], sqn_rep[:, sl], AF.Identity,
                                         bias=sqmT[:, t:t + 1])
                    nc.vector.scalar_tensor_tensor(negd[:, sl], pd[:], 2.0, trow[:],
                                                   op0=AluOpType.mult,
                                                   op1=AluOpType.subtract)

                vals = sml.tile([128, 40], F32, tag="vals")
                idxs = sml.tile([128, 40], U32, tag="idxs")
                for r in range(5):
                    v8 = vals[:, r * 8:(r + 1) * 8]
                    nc.vector.max(v8, negd[:])
                    nc.vector.max_index(idxs[:, r * 8:(r + 1) * 8], v8, negd[:])
                    if r < 4:
                        nc.vector.match_replace(negd[:], v8, negd[:], NEG_INF)

                # wrap lists for the gathers
                idxf = sml.tile([128, K], F32, tag="idxf")
                nc.vector.tensor_copy(idxf[:], idxs[:, 0:K])
                L = sml.tile([128, 288], I16, tag="L")
                L3 = L[:].rearrange("p (k e) -> p k e", k=K, e=8)
                for a in range(8):
                    pw = psr.tile([128, K], F32, tag="pwrap")
                    nc.tensor.matmul(pw[:], Cw[:, a * 128:(a + 1) * 128], idxf[:],
                                     start=True, stop=True)
                    nc.vector.tensor_copy(L3[:, :, a], pw[:])

                pre2_t = med.tile([128, K, DM], F32, tag="pre2t")
                w_t = med1.tile([128, K, DM], F32, tag="wt")
                res_ps = psr.tile([128, DM], F32, tag="resps")

                for third in range(3):
                    ksl = slice(third * KT * 8, (third + 1) * KT * 8)
                    kge = gat.tile([128, KT * 128, 2], F32, tag="kge")
                    nc.gpsimd.ap_gather(kge[:], kge_src[:], L[:, ksl],
                                        channels=128, num_elems=N, d=2,
                                        num_idxs=KT * 128)
                    vgt = gat1.tile([128, KT * 128], F32, tag="vgt")
                    nc.gpsimd.ap_gather(vgt[:], v_src[:], L[:, ksl],
                                        channels=128, num_elems=N, d=1,
                                        num_idxs=KT * 128)

                    for gl in range(KT // GRP):
                        k0 = third * KT + gl * GRP          # absolute k
                        lsl = slice(gl * GRP * 128, (gl + 1) * GRP * 128)
                        # h = relu(ep_m + eneg_g)
                        hp = ps.tile([128, 512], F32, tag="ps")
                        nc.tensor.matmul(hp[:], I128, kge[:, lsl, 1],
                                         start=True, stop=False)
                        for k2 in range(GRP):
                            nc.tensor.matmul(hp[:, k2 * 128:(k2 + 1) * 128],
                                             I128, ep[:, tsl],
                                             start=False, stop=(k2 == GRP - 1),
                                             skip_group_check=True)
                        h_sb = sml.tile([128, 512], F32, tag="hsb")
                        nc.scalar.activation(h_sb[:], hp[:], AF.Relu)

                        # u = relu(Pd h + qgp_m + kgneg_g)
                        gp = ps.tile([128, 512], F32, tag="ps")
                        nc.tensor.matmul(gp[:], PdT, h_sb[:],
                                         start=True, stop=False)
                        nc.tensor.matmul(gp[:], I128, kge[:, lsl, 0],
                                         start=False, stop=False,
                                         skip_group_check=True)
                        for k2 in range(GRP):
                            nc.tensor.matmul(gp[:, k2 * 128:(k2 + 1) * 128],
                                             I128, qgp[:, tsl],
                                             start=False, stop=(k2 == GRP - 1),
                                             skip_group_check=True)
                        u_sb = sml.tile([128, 512], F32, tag="usb")
                        nc.scalar.activation(u_sb[:], gp[:], AF.Relu)

                        # pair-major: w = h^T d2 + b2d + vg^T ; pre2 = u^T g2 + b2g
                        wp = ps.tile([128, 512], F32, tag="ps")
                        pp = ps.tile([128, 512], F32, tag="ps")
                        for k2 in range(GRP):
                            osl = slice(k2 * 128, (k2 + 1) * 128)
                            lk = (gl * GRP + k2) * 128
                            nc.tensor.matmul(wp[:, osl], h_sb[:, osl], d2T,
                                             start=True, stop=False)
                            nc.tensor.matmul(wp[:, osl], ones1, b2d_row,
                                             start=False, stop=False,
                                             skip_group_check=True)
                            nc.tensor.matmul(wp[:, osl], vgt[:, lk:lk + 128],
                                             I128, start=False, stop=True,
                                             skip_group_check=True)
                            nc.tensor.matmul(pp[:, osl], u_sb[:, osl], g2T,
                                             start=True, stop=False)
                            nc.tensor.matmul(pp[:, osl], ones1, b2g_row,
                                             start=False, stop=True,
                                             skip_group_check=True)
                        w_flat = w_t[:].rearrange("p k c -> p (k c)")
                        p2_flat = pre2_t[:].rearrange("p k c -> p (k c)")
                        gmem = slice(k0 * 128, (k0 + GRP) * 128)
                        nc.scalar.activation(w_flat[:, gmem], wp[:], AF.Identity)
                        nc.scalar.activation(p2_flat[:, gmem], pp[:], AF.Identity)

                # denom + reciprocal per tile
                den = sml.tile([128, K], F32, tag="den")
                nc.vector.tensor_reduce(den[:], pre2_t[:], mybir.AxisListType.X,
                                        AluOpType.add, apply_absolute_value=True)
                nc.vector.tensor_scalar_add(den[:], den[:], float(DM) * 1e-5)
                rec = sml.tile([128, K], F32, tag="rec")
                nc.vector.reciprocal(rec[:], den[:])

                # attn = pre2*rec (gpsimd), prod = attn*w (dve), res += prod (pe)
                for g in range(K // GRP):
                    k0 = g * GRP
                    attn_g = sml.tile([128, GRP, DM], F32, tag="attng")
                    rec_b = rec[:, k0:k0 + GRP].to_broadcast([128, GRP, DM])
                    nc.gpsimd.tensor_tensor(attn_g[:], pre2_t[:, k0:k0 + GRP, :],
                                            rec_b, AluOpType.mult)
                    nc.sync.dma_start(
                        attn_o[:].rearrange("(t m) k c -> t m k c", m=128)
                        [t, :, k0:k0 + GRP, :], attn_g[:])
                    prod_g = sml.tile([128, GRP, DM], F32, tag="prodg")
                    nc.vector.tensor_tensor(prod_g[:], attn_g[:],
                                            w_t[:, k0:k0 + GRP, :], AluOpType.mult)
                    for k2 in range(GRP):
                        nc.tensor.matmul(res_ps[:], I128, prod_g[:, k2, :],
                                         start=(k0 + k2 == 0),
                                         stop=(k0 + k2 == K - 1),
                                         skip_group_check=True)

                # res1 tile -> channel-major column block
                res_pm = sml.tile([128, DM], F32, tag="respm")
                nc.vector.tensor_copy(res_pm[:], res_ps[:])
                pt = psr.tile([128, DM], F32, tag="ptr")
                nc.tensor.matmul(pt[:], res_pm[:], I128, start=True, stop=True)
                nc.vector.tensor_copy(res1_cm[:, tsl], pt[:])

            # ---------------- mtlinear x2 ----------------
            def mtlinear(res_cm, cwT_dram, cb_s, mf_in, mf_out, resid_cm):
                part = sml.tile([DM, 1], F32, tag="part")
                nc.vector.tensor_reduce(part[:], res_cm[:], mybir.AxisListType.X,
                                        AluOpType.add)
                pvec = sml.tile([DM, B], F32, tag="pvec")
                nc.vector.tensor_tensor(pvec[:], pmask,
                                        part[:].to_broadcast([DM, B]),
                                        AluOpType.mult)
                nc.sync.dma_start(mf_in[:], pvec[:])
                nc.gpsimd.collective_compute(
                    "AllReduce", AluOpType.add, replica_groups=[list(range(8))],
                    ins=[mf_in[:]], outs=[mf_out[:]])
                mfv = sml.tile([DM, B], F32, tag="mfv")
                nc.sync.dma_start(mfv[:], mf_out[:])
                nc.vector.tensor_tensor(mfv[:], mfv[:], pmask, AluOpType.mult)
                mf = sml.tile([DM, 1], F32, tag="mf")
                nc.vector.tensor_reduce(mf[:], mfv[:], mybir.AxisListType.X,
                                        AluOpType.add)

                wc_ps = psr.tile([128, 128], F32, tag="wcps")
                for blk in range(8):
                    stg = gat.tile([128, 2048], F32, tag="kge")
                    nc.sync.dma_start(stg[:], cwT_dram[:, blk * 2048:(blk + 1) * 2048])
                    for s in range(16):
                        o = blk * 16 + s
                        nc.tensor.matmul(wc_ps[:, o:o + 1],
                                         stg[:, s * 128:(s + 1) * 128], mf[:],
                                         start=True, stop=True,
                                         skip_group_check=True)
                wcT = sml.tile([128, 128], F32, tag="wcT")
                nc.vector.tensor_tensor(wcT[:], wc_ps[:], cb_s, AluOpType.add)
                wden = sml.tile([128, 1], F32, tag="wden")
                nc.vector.tensor_reduce(wden[:], wcT[:], mybir.AxisListType.X,
                                        AluOpType.add, apply_absolute_value=True)
                nc.vector.tensor_scalar_add(wden[:], wden[:], float(DM) * 1e-5)
                wrec = sml.tile([128, 1], F32, tag="wrec")
                nc.vector.reciprocal(wrec[:], wden[:])
                nc.vector.tensor_scalar_mul(wcT[:], wcT[:], wrec[:, 0:1])

                out_cm = med.tile([DM, NLOC], F32, tag="big")
                for c in range(2):
                    sl = slice(c * 512, (c + 1) * 512)
                    p = ps.tile([128, 512], F32, tag="ps")
                    nc.tensor.matmul(p[:], wcT[:], res_cm[:, sl],
                                     start=True, stop=False)
                    nc.tensor.matmul(p[:], I128, resid_cm[:, sl],
                                     start=False, stop=True, skip_group_check=True)
                    nc.vector.tensor_copy(out_cm[:, sl], p[:])
                return out_cm

            res2_cm = mtlinear(res1_cm, cw2T_d, cb2, mf2_in, mf2_out, x_loc)
            res3_cm = mtlinear(res2_cm, cw3T_d, cb3, mf3_in, mf3_out, res2_cm)

            # ---------------- output transpose ----------------
            for t in range(NT):
                pt = psr.tile([128, DM], F32, tag="ptr")
                nc.tensor.matmul(pt[:], res3_cm[:, t * 128:(t + 1) * 128], I128,
                                 start=True, stop=True)
                rt = sml.tile([128, DM], F32, tag="rt")
                nc.vector.tensor_copy(rt[:], pt[:])
                nc.sync.dma_start(res_o[t * 128:(t + 1) * 128, :], rt[:])

    nc.compile()
    return nc


def _prep_host(features, xyz, fc1_w, fc1_b, wq, wk, wv,
               delta_w1, delta_b1, delta_w2, delta_b2,
               gamma_w1, gamma_b1, gamma_w2, gamma_b2,
               fc2_cw, fc2_cb, fc3_cw, fc3_cb):
    f32 = np.float32
    xyz = np.asarray(xyz, f32)
    features = np.asarray(features, f32)
    g1 = np.asarray(gamma_w1, f32)
    wmats = np.concatenate([
        -(g1 @ np.asarray(wk, f32)).T,
        (g1 @ np.asarray(wq, f32)).T,
        np.asarray(wv, f32).T,
        (g1 @ np.asarray(delta_w2, f32)).T,
        np.asarray(delta_w2, f32).T,
        np.asarray(gamma_w2, f32).T,
        np.asarray(fc2_cb, f32).reshape(DM, DM).T,
        np.asarray(fc3_cb, f32).reshape(DM, DM).T,
        np.eye(DM, dtype=f32),
    ], axis=1)
    wrows = np.concatenate([
        np.asarray(delta_b2, f32).reshape(1, DM),
        np.asarray(gamma_b2, f32).reshape(1, DM),
        np.ones((1, DM), f32),
    ], axis=1)
    consts = {
        "fc1_wT": np.ascontiguousarray(np.asarray(fc1_w, f32).T),
        "dw1_negT": np.ascontiguousarray(-np.asarray(delta_w1, f32).T),
        "wmats": np.ascontiguousarray(wmats),
        "wrows": np.ascontiguousarray(wrows),
        "cw2T": np.ascontiguousarray(np.asarray(fc2_cw, f32).T) / f32(N),
        "cw3T": np.ascontiguousarray(np.asarray(fc3_cw, f32).T) / f32(N),
    }
    wcols_base = np.zeros((DM, 16), f32)
    wcols_base[:, 0] = np.asarray(fc1_b, f32)
    wcols_base[:, 1] = (np.asarray(gamma_b1, f32) + g1 @ np.asarray(delta_b2, f32))
    wcols_base[:, 2] = np.asarray(delta_b1, f32)
    Cw = np.zeros((8, 128, 128), f32)
    for a in range(8):
        for p in range(128):
            Cw[a, 16 * a + p % 16, p] = 1.0
    consts["Cw"] = Cw

    in_maps = []
    for core in range(8):
        b, h = core // 2, core % 2
        xb = xyz[b]
        xyzT_b = np.ascontiguousarray(xb.T)
        sq = (xb[:, 0] * xb[:, 0] + xb[:, 1] * xb[:, 1]) + xb[:, 2] * xb[:, 2]
        loc = slice(h * NLOC, (h + 1) * NLOC)
        m = dict(consts)
        wc = wcols_base.copy()
        wc[:, 3:3 + NT] = sq[loc].reshape(NT, 128).T
        wc[:, 11 + b] = 1.0
        m["wcols"] = wc
        m["xyzT"] = xyzT_b
        m["xyzTl"] = np.ascontiguousarray(xyzT_b[:, loc])
        m["sq_row"] = sq.reshape(1, N)
        ftT = np.ascontiguousarray(features[b].T)
        m["featT"] = ftT
        m["featTl"] = np.ascontiguousarray(ftT[:, loc])
        in_maps.append(m)
    return in_maps


def kernel(**inputs):
    if "nc" not in _CACHE:
        _CACHE["nc"] = _build()
    nc = _CACHE["nc"]
    in_maps = _prep_host(**inputs)
    res = bass_utils.run_bass_kernel_spmd(nc, in_maps, list(range(8)))
    attn = np.empty((B, N, K, DM), np.float32)
    out = np.empty((B, N, DM), np.float32)
    for core in range(8):
        b, h = core // 2, core % 2
        r = res.results[core]
        attn[b, h * NLOC:(h + 1) * NLOC] = r["attn_o"]
        out[b, h * NLOC:(h + 1) * NLOC] = r["res_o"]
    return out, attn
